# revision 4
# baseline (speedup 1.0000x reference)
"""BinSAGE on 8 TRN2 cores — v4.

vs v3:
  - One-hot matrices are host-materialized (the segment structure is static),
    pre-scaled by 1/deg(dst), and streamed alongside the layer-1 messages /
    as a layer-2 side stream. This removes every per-chunk DVE op and the
    whole per-tile mean-scale chain (rank-1 matmul + copy + multiply).
  - Balanced node->tile permutation (degree-aware) equalizes per-tile edge
    counts: fewer padded chunks, and the layer-2 class margins shrink.
  - AllGather pieces (8, 21, 21) layer-1 tiles: the layer-2 SWDGE gather
    stream (the critical resource) starts after only 8 layer-1 tiles.
  - Layer-2 output tail is fused into the last accumulation pass.
"""

import numpy as np
import ml_dtypes

import concourse.bass as bass
import concourse.bacc as bacc
import concourse.mybir as mybir
import concourse.tile as tile
from concourse import bass_utils

BF16 = ml_dtypes.bfloat16
P = 128
N_CORES = 8
ROW = 128
GC = 8
PIECE_TILES = (8, 21, 21)
NPIECE = len(PIECE_TILES)


class Cfg:
    def __init__(self, n_nodes, in_dim, hid, out_dim, tiles_per_core):
        self.n_nodes = n_nodes
        self.in_dim = in_dim
        self.hid = hid
        self.out_dim = out_dim
        self.tiles_per_core = tiles_per_core
        self.span = tiles_per_core * P
        self.n_pad = self.span * N_CORES
        assert sum(PIECE_TILES) == tiles_per_core
        self.piece_rows = [pt * P for pt in PIECE_TILES]
        self.piece_off = np.concatenate([[0], np.cumsum(self.piece_rows)])
        self.piece_tab_rows = [pr * N_CORES for pr in self.piece_rows]
        assert all(r <= 32767 for r in self.piece_tab_rows)


FULL_CFG = Cfg(n_nodes=50000, in_dim=96, hid=128, out_dim=64, tiles_per_core=50)


def _wrap16(v):
    n = len(v)
    return np.ascontiguousarray(np.tile(v.reshape(n // 16, 16).T, (8, 1)))


def balanced_perm(dst, n_nodes, n_pad):
    """Degree-aware snake assignment of nodes to 128-slot tiles.
    Returns newpos[node] -> permuted position in [0, n_pad)."""
    deg = np.bincount(dst, minlength=n_pad).astype(np.int64)[:n_nodes]
    order = np.argsort(-deg, kind="stable")
    tiles = n_pad // P
    i = np.arange(n_nodes) % (2 * tiles)
    tile_of = np.where(i < tiles, i, 2 * tiles - 1 - i)
    newpos = np.empty(n_nodes, np.int64)
    slot = np.zeros(tiles, np.int64)
    for i, n in enumerate(order):
        t = tile_of[i]
        newpos[n] = t * P + slot[t]
        slot[t] += 1
    assert slot.max() <= P
    return newpos


class Sched:
    def __init__(self, eff_k1, eff_k2):
        self.eff_k1 = eff_k1
        self.eff_k2 = eff_k2
        self.off1 = np.zeros(len(eff_k1) + 1, np.int64)
        self.off1[1:] = np.cumsum(eff_k1)
        self.SD1 = int(self.off1[-1])
        self.K1M = int(eff_k1.max())
        self.off2 = np.zeros((NPIECE, eff_k2.shape[1] + 1), np.int64)
        self.off2[:, 1:] = np.cumsum(eff_k2, axis=1)
        self.SL = [int(self.off2[c, -1]) for c in range(NPIECE)]
        self.off_d = np.zeros(eff_k2.shape[1] + 1, np.int64)
        self.off_d[1:] = np.cumsum(eff_k2.sum(axis=0))
        self.SD2 = int(self.off_d[-1])


def preprocess(x, edge_index, w1_l, b1, w1_r, w2_l, b2, w2_r, cfg):
    src0 = np.asarray(edge_index[0]).astype(np.int64)
    dst0 = np.asarray(edge_index[1]).astype(np.int64)
    x = np.asarray(x, np.float32)
    tpc = cfg.tiles_per_core
    n_tiles_total = N_CORES * tpc

    newpos = balanced_perm(dst0, cfg.n_nodes, cfg.n_pad)
    src_p = newpos[src0]          # permuted positions
    dst_p = newpos[dst0]
    g = dst_p // P

    deg = np.bincount(dst_p, minlength=cfg.n_pad).astype(np.float32)
    rdeg = (1.0 / np.maximum(deg, 1.0)).astype(np.float32)
    # per-edge one-hot value = 1/deg(dst), bf16
    ohval = rdeg[dst_p].astype(BF16)

    # ---------------- layer-1 schedule (single class) ----------------
    order1 = np.lexsort((src_p, g))
    g1 = g[order1]
    src1o = src0[order1]
    dloc1_v = (dst_p[order1] % P).astype(np.int64)
    ohval1 = ohval[order1]
    cnt = np.bincount(g1, minlength=n_tiles_total).astype(np.int64)
    eff_k1 = np.maximum(
        np.ceil(cnt.reshape(N_CORES, tpc).max(axis=0) / P).astype(np.int64), 1)

    # ---------------- layer-2 schedule (class = piece of permuted src) --
    c_s = src_p // cfg.span
    off_s = src_p % cfg.span
    pc_s = np.searchsorted(cfg.piece_off, off_s, side="right") - 1
    toff = off_s - cfg.piece_off[pc_s]
    prows = np.asarray(cfg.piece_rows)[pc_s]
    row2 = c_s * prows + toff
    order2 = np.lexsort((row2, pc_s, g))
    g2 = g[order2]
    row2_s = row2[order2]
    pc2 = pc_s[order2]
    dloc2_v = (dst_p[order2] % P).astype(np.int64)
    ohval2 = ohval[order2]

    cnt2 = np.zeros((NPIECE, n_tiles_total), np.int64)
    for c in range(NPIECE):
        cnt2[c] = np.bincount(g2[pc2 == c], minlength=n_tiles_total)
    eff_k2 = np.ceil(
        cnt2.reshape(NPIECE, N_CORES, tpc).max(axis=1) / P).astype(np.int64)
    eff_k2[0] = np.maximum(eff_k2[0], 1)
    sched = Sched(eff_k1, eff_k2)

    offs1 = np.zeros(n_tiles_total + 1, np.int64)
    offs1[1:] = np.cumsum(cnt)
    pos1 = np.arange(len(order1)) - offs1[g1]

    cnt2_tot = cnt2.sum(axis=0)
    offs2 = np.zeros(n_tiles_total + 1, np.int64)
    offs2[1:] = np.cumsum(cnt2_tot)
    pos2 = np.arange(len(order2)) - offs2[g2]
    cnt2_before = np.concatenate(
        [np.zeros((1, n_tiles_total), np.int64), np.cumsum(cnt2, axis=0)[:-1]])
    posc = pos2 - cnt2_before[pc2, g2]

    K1M = sched.K1M
    K2M = [int(max(eff_k2[c].max(), 1)) for c in range(NPIECE)]

    # fused layer-1 stream: [tile-chunk slots, 96 msg | 128 scaled one-hot]
    SW = cfg.in_dim + P
    s1_full = np.zeros((n_tiles_total, K1M * P, SW), dtype=BF16)
    s1_full[g1, pos1, : cfg.in_dim] = x[src1o].astype(BF16)
    s1_full[g1, pos1, cfg.in_dim + dloc1_v] = ohval1

    # layer-2: int16 gather rows + scaled one-hot stream
    idx2 = [np.zeros((n_tiles_total, K2M[c] * P), dtype=np.int16)
            for c in range(NPIECE)]
    oh2 = np.zeros((n_tiles_total,
                    int(eff_k2.sum(axis=0).max()) * P, P), dtype=BF16)
    kcum = np.zeros(n_tiles_total, np.int64)
    for c in range(NPIECE):
        m = pc2 == c
        idx2[c][g2[m], posc[m]] = row2_s[m].astype(np.int16)
    # oh2 slot position within the tile's full (class-ordered) chunk layout
    eff_before = np.concatenate(
        [np.zeros((1, tpc), np.int64), np.cumsum(eff_k2, axis=0)[:-1]])
    t_loc = g2 % tpc
    slot2 = eff_before[pc2, t_loc] * P + posc
    oh2[g2, slot2, dloc2_v] = ohval2

    sgn = lambda w: np.sign(np.asarray(w, dtype=np.float32))
    w1lt = np.concatenate([sgn(w1_l).T, np.asarray(b1, np.float32)[None, :]],
                          0).astype(BF16)
    w1rt = np.ascontiguousarray(sgn(w1_r).T).astype(BF16)
    w2lt = np.ascontiguousarray(sgn(w2_l).T).astype(BF16)
    w2rt = np.ascontiguousarray(sgn(w2_r).T).astype(BF16)
    ib2 = np.concatenate(
        [np.eye(cfg.out_dim, dtype=np.float32),
         np.asarray(b2, np.float32)[None, :]], 0).astype(BF16)

    # permuted x^T for the self term
    xpt = np.zeros((cfg.in_dim, cfg.n_pad), dtype=BF16)
    xpt[:, newpos] = x.T.astype(BF16)

    in_maps = []
    for c in range(N_CORES):
        s1_parts = []
        idx_parts = [[] for _ in range(NPIECE)]
        oh2_parts = []
        for t in range(tpc):
            gt = c * tpc + t
            k1 = int(eff_k1[t])
            s1_parts.append(
                s1_full[gt, : k1 * P].reshape(k1, P, SW).transpose(1, 0, 2))
            ktot = int(eff_k2[:, t].sum())
            oh2_parts.append(
                oh2[gt, : ktot * P].reshape(ktot, P, P).transpose(1, 0, 2))
            for cl in range(NPIECE):
                kc = int(eff_k2[cl][t])
                idx_parts[cl].append(idx2[cl][gt, : kc * P])
        entry = {
            "s1": np.ascontiguousarray(np.concatenate(s1_parts, axis=1)),
            "oh2": np.ascontiguousarray(np.concatenate(oh2_parts, axis=1)),
            "xt": np.ascontiguousarray(
                xpt[:, c * cfg.span:(c + 1) * cfg.span]),
            "w1lt": w1lt, "w1rt": w1rt, "w2lt": w2lt, "w2rt": w2rt,
            "ib2": ib2,
        }
        for cl in range(NPIECE):
            v = np.concatenate(idx_parts[cl])
            entry[f"idx2_{cl}"] = _wrap16(
                v if len(v) else np.zeros(P, np.int16))
        in_maps.append(entry)
    return in_maps, sched, newpos


def build_program(cfg, sched):
    tpc = cfg.tiles_per_core
    NBUF = 10                     # rotating L2 gather-call buffers per class
    NB1 = 4                       # rotating L1 stream buffers
    NBO = 6                       # rotating L2 one-hot stream buffers
    NB = 3
    SD1, SD2 = sched.SD1, sched.SD2
    K1M = sched.K1M

    dt = mybir.dt
    f32, bf, i16 = dt.float32, dt.bfloat16, dt.int16
    IN, HID, OUT = cfg.in_dim, cfg.hid, cfg.out_dim
    SW = IN + P

    nc = bacc.Bacc("TRN2", target_bir_lowering=False, debug=False,
                   num_devices=N_CORES)

    s1 = nc.dram_tensor("s1", [P, SD1, SW], bf, kind="ExternalInput")
    oh2d = nc.dram_tensor("oh2", [P, SD2, P], bf, kind="ExternalInput")
    xt = nc.dram_tensor("xt", [IN, cfg.span], bf, kind="ExternalInput")
    idx2_d = [nc.dram_tensor(f"idx2_{c}", [P, max(sched.SL[c], 1) * 8], i16,
                             kind="ExternalInput") for c in range(NPIECE)]
    w1lt = nc.dram_tensor("w1lt", [IN + 1, HID], bf, kind="ExternalInput")
    w1rt = nc.dram_tensor("w1rt", [IN, HID], bf, kind="ExternalInput")
    w2lt = nc.dram_tensor("w2lt", [HID, OUT], bf, kind="ExternalInput")
    w2rt = nc.dram_tensor("w2rt", [HID, OUT], bf, kind="ExternalInput")
    ib2 = nc.dram_tensor("ib2", [OUT + 1, OUT], bf, kind="ExternalInput")
    outd = nc.dram_tensor("out", [cfg.span, OUT], f32, kind="ExternalOutput")

    AF = mybir.ActivationFunctionType
    OP = mybir.AluOpType

    with tile.TileContext(nc) as tc:
        with tc.tile_pool(name="res", bufs=1) as res, \
             tc.tile_pool(name="msgp", bufs=1) as msgp, \
             tc.tile_pool(name="xtp", bufs=3) as xtp, \
             tc.tile_pool(name="scp", bufs=3) as scp, \
             tc.tile_pool(name="ps_agg", bufs=2, space="PSUM") as ps_agg, \
             tc.tile_pool(name="ps_o", bufs=2, space="PSUM") as ps_o, \
             tc.tile_pool(name="ps_y", bufs=2, space="PSUM") as ps_y, \
             tc.tile_pool(name="dramp", bufs=1, space="DRAM") as dramp:

            # ---------------- resident data ----------------
            idx2_sb = []
            for c in range(NPIECE):
                t_ = res.tile([P, max(sched.SL[c], 1) * 8], i16,
                              name=f"idx2sb_{c}")
                nc.sync.dma_start(t_[:], idx2_d[c][:])
                idx2_sb.append(t_)
            w1lt_sb = res.tile([IN + 1, HID], bf, name="w1lt_sb")
            nc.sync.dma_start(w1lt_sb[:], w1lt[:])
            w1rt_sb = res.tile([IN, HID], bf, name="w1rt_sb")
            nc.sync.dma_start(w1rt_sb[:], w1rt[:])
            w2lt_sb = res.tile([HID, OUT], bf, name="w2lt_sb")
            nc.sync.dma_start(w2lt_sb[:], w2lt[:])
            w2rt_sb = res.tile([HID, OUT], bf, name="w2rt_sb")
            nc.sync.dma_start(w2rt_sb[:], w2rt[:])
            ib2_sb = res.tile([OUT + 1, OUT], bf, name="ib2_sb")
            nc.sync.dma_start(ib2_sb[:], ib2[:])

            ht_tiles = [res.tile([HID, P], bf, name=f"ht{t}")
                        for t in range(tpc)]
            acc_tiles = [res.tile([OUT, P], f32, name=f"acc{t}")
                         for t in range(tpc)]

            m_cls = [[msgp.tile([P, GC, ROW], bf, name=f"m{c}_{i}")
                      for i in range(NBUF)] for c in range(NPIECE)]
            s1bufs = [msgp.tile([P, K1M, SW], bf, name=f"s1_{i}")
                      for i in range(NB1)]
            K2CM = int(max(sched.eff_k2.max(), 1))
            ohbufs = [msgp.tile([P, K2CM, P], bf, name=f"oh2_{i}")
                      for i in range(NBO)]
            aggs1 = [msgp.tile([IN + 1, P], bf, name=f"aggs1_{i}")
                     for i in range(NB)]
            aggs2 = [msgp.tile([OUT + 1, P], bf, name=f"aggs2_{i}")
                     for i in range(NB)]
            for i in range(NB):
                nc.gpsimd.memset(aggs1[i][IN:IN + 1, :], 1.0)
                nc.gpsimd.memset(aggs2[i][OUT:OUT + 1, :], 1.0)

            y2in = [dramp.tile([cfg.piece_rows[p], OUT], bf, name=f"y2in{p}")
                    for p in range(NPIECE)]
            y2c = [dramp.tile([cfg.piece_tab_rows[p], OUT], bf,
                              name=f"y2c{p}", addr_space="Shared")
                   for p in range(NPIECE)]
            y2f = [dramp.tile([cfg.piece_tab_rows[p], ROW], bf,
                              name=f"y2f{p}") for p in range(NPIECE)]

            piece_of_tile = []
            for p, nt in enumerate(PIECE_TILES):
                piece_of_tile += [p] * nt
            tile_in_piece = []
            for nt in PIECE_TILES:
                tile_in_piece += list(range(nt))

            # ---------------- layer 1 (software-pipelined tail) ----------
            # Per-iteration: chunk matmuls for tile t, transform for t-1, y2
            # projection + exchange for t-2 — so the in-order PE queue never
            # waits on a Scalar-engine round trip (ab copy / relu).
            xts = {}

            def stage_a(u):
                ab = aggs1[u % NB]
                hps = ps_o.tile([HID, P], f32, tag="hps")
                nc.tensor.matmul(out=hps[:], lhsT=w1lt_sb[:], rhs=ab[:],
                                 start=True, stop=False)
                nc.tensor.matmul(out=hps[:], lhsT=w1rt_sb[:], rhs=xts[u][:],
                                 start=False, stop=True)
                nc.scalar.activation(out=ht_tiles[u][:], in_=hps[:],
                                     func=AF.Relu)

            def stage_b(u):
                y2ps = ps_y.tile([P, OUT], f32, tag="y2ps")
                nc.tensor.matmul(out=y2ps[:], lhsT=ht_tiles[u][:],
                                 rhs=w2lt_sb[:], start=True, stop=True)
                ysb = scp.tile([P, OUT], bf, tag="ysb")
                nc.scalar.activation(out=ysb[:], in_=y2ps[:], func=AF.Copy)
                pc, tin = piece_of_tile[u], tile_in_piece[u]
                nc.sync.dma_start(y2in[pc][tin * P:(tin + 1) * P, :], ysb[:])
                if tin == PIECE_TILES[pc] - 1:
                    nc.gpsimd.collective_compute(
                        "AllGather", OP.bypass,
                        replica_groups=[list(range(N_CORES))],
                        ins=[y2in[pc][:].opt()],
                        outs=[y2c[pc][:].opt()],
                    )
                    nc.sync.dma_start(y2f[pc][:, 0:OUT], y2c[pc][:])

            for t in range(tpc):
                k1 = int(sched.eff_k1[t])
                c0 = int(sched.off1[t])
                m1 = s1bufs[t % NB1]
                nc.sync.dma_start(m1[:, 0:k1, :], s1[:, c0:c0 + k1, :])
                xt_t = xtp.tile([IN, P], bf, tag="xt")
                nc.sync.dma_start(xt_t[:], xt[:, t * P:(t + 1) * P])
                xts[t] = xt_t

                agg = ps_agg.tile([IN, P], f32, tag="agg")
                for j in range(k1):
                    nc.tensor.matmul(
                        out=agg[:], lhsT=m1[:, j, 0:IN],
                        rhs=m1[:, j, IN:SW],
                        start=(j == 0), stop=(j == k1 - 1))
                ab = aggs1[t % NB]
                nc.scalar.activation(out=ab[0:IN, :], in_=agg[:], func=AF.Copy)
                if t >= 1:
                    stage_a(t - 1)
                if t >= 2:
                    stage_b(t - 2)
            stage_a(tpc - 1)
            stage_b(tpc - 2)
            stage_b(tpc - 1)

            # ---------------- layer 2: 3 passes, tail fused in last ------
            emitted = [0] * NPIECE

            def ensure(cls, upto_call):
                ncalls_tot = (sched.SL[cls] + GC - 1) // GC
                upto_call = min(upto_call, ncalls_tot - 1)
                while emitted[cls] <= upto_call:
                    c = emitted[cls]
                    ncall = min(GC, sched.SL[cls] - c * GC)
                    num = ncall * P
                    dest = m_cls[cls][c % NBUF]
                    nc.gpsimd.dma_gather(
                        out_ap=dest[:, 0:ncall, :],
                        in_ap=y2f[cls][:],
                        idxs_ap=idx2_sb[cls][:, c * (GC * 8):
                                             c * (GC * 8) + num // 16],
                        num_idxs=num,
                        num_idxs_reg=num,
                        elem_size=ROW,
                    )
                    emitted[cls] += 1

            oh_it = 0
            for cls in range(NPIECE):
                if sched.SL[cls] == 0:
                    continue
                for t in range(tpc):
                    kc = int(sched.eff_k2[cls][t])
                    if kc == 0:
                        continue
                    # prefetch this class's stream a few tiles ahead
                    tp = min(t + 3, tpc - 1)
                    ensure(cls, (int(sched.off2[cls][tp])
                                 + max(int(sched.eff_k2[cls][tp]), 1) - 1) // GC)
                    kbefore = int(sched.eff_k2[:cls, t].sum())
                    ohb = ohbufs[oh_it % NBO]
                    oh_it += 1
                    col0 = int(sched.off_d[t]) + kbefore
                    nc.sync.dma_start(ohb[:, 0:kc, :],
                                      oh2d[:, col0:col0 + kc, :])
                    agg = ps_agg.tile([OUT, P], f32, tag="agg")
                    for j in range(kc):
                        cpos = int(sched.off2[cls][t]) + j
                        mb = m_cls[cls][(cpos // GC) % NBUF]
                        nc.tensor.matmul(
                            out=agg[:], lhsT=mb[:, cpos % GC, 0:OUT],
                            rhs=ohb[:, j, :],
                            start=(j == 0), stop=(j == kc - 1))
                    if cls == 0:
                        nc.scalar.activation(out=acc_tiles[t][:], in_=agg[:],
                                             func=AF.Copy)
                    else:
                        nc.vector.tensor_tensor(acc_tiles[t][:],
                                                acc_tiles[t][:], agg[:],
                                                OP.add)
                    if cls == NPIECE - 1:
                        ab = aggs2[t % NB]
                        nc.vector.tensor_copy(ab[0:OUT, :], acc_tiles[t][:])
                        ops_ = ps_o.tile([P, OUT], f32, tag="hps")
                        nc.tensor.matmul(out=ops_[:], lhsT=ht_tiles[t][:],
                                         rhs=w2rt_sb[:], start=True,
                                         stop=False)
                        nc.tensor.matmul(out=ops_[:], lhsT=ab[:],
                                         rhs=ib2_sb[:], start=False,
                                         stop=True)
                        osb = scp.tile([P, OUT], f32, tag="osb")
                        nc.vector.tensor_copy(osb[:], ops_[:])
                        nc.sync.dma_start(outd[t * P:(t + 1) * P, :], osb[:])

    nc.compile()
    return nc


def run(inputs, cfg, trace=False):
    in_maps, sched, newpos = preprocess(cfg=cfg, **inputs)
    nc = build_program(cfg, sched)
    res = bass_utils.run_bass_kernel_spmd(
        nc, in_maps, list(range(N_CORES)), trace=trace)
    outs = [res.results[c]["out"] for c in range(N_CORES)]
    full_perm = np.concatenate(outs, axis=0)
    full = full_perm[newpos]                  # un-permute to node order
    return np.ascontiguousarray(full.astype(np.float32)), res


def kernel(**inputs):
    out, _ = run(inputs, FULL_CFG, trace=False)
    return out
